# revision 44
# baseline (speedup 1.0000x reference)
"""Trainium2 Bass kernel for batched single-query attention (Luong-style).

  scores[b, t] = dec_hid[b] . enc_hid_states[b, t]      # [B, T]
  align        = softmax(scores, axis=1)
  c_t[b, d]    = sum_t align[b, t] * enc_hid_states[b, t, d]

Shapes: enc_hid_states [32, 8192, 256] f32, dec_hid [32, 256] f32.
Sharding: data-parallel over batch; 4 batches per core on 8 cores. The
host pre-casts both inputs to fp16 while sharding (numerically identical
to the on-device casting DMA the previous version used, and it frees the
GPSIMD engine for compute). Each core emits per-chunk softmax partials
(chunk = 2048 consecutive t of one batch); the host combines the 4
chunks of each batch with an exact log-sum-exp reduction in float64.

Per-core pipeline (v6). t-chunks tile each batch: 1024-t chunks at the
core's schedule head/tail (fast pipeline fill/drain), 2048-t in the
middle. A chunk with nj j-tiles is laid out [p=128, j=nj, d=256] with
t = t0 + p*nj + j so each partition reads one contiguous nj*512 B HBM
run (128 large DMA descriptors, full modeled bus rate). Per chunk the
scores dot-products are split across three engines to match DMA pace:
  - multiplies by broadcast dec: two DVE 3D tensor_tensors (all-fp16
    2x_1p mode; the ACT-bound j-tiles get a separate early one) and a
    GPSIMD tensor_tensor for 5/16 j-tiles
  - reduction over d: 4 ACT Copy-with-accum + 12 DVE
    tensor_scalar(+accum_out) at the 4x_2p mode ([P,1] accum is
    dtype-exempt)
  - chunk max: DVE tensor_reduce + GPSIMD partition all-reduce (max) +
    GPSIMD negate, all deferred one chunk so ACT-copy jitter never
    stalls the DVE stream; ACT Exp with bias=-m writes fp16 probs
  - nj accumulating PE matmuls (lhsT=probs column [128,1], rhs=enc
    j-tile [128,256], fp16 full-rate) produce the chunk context in
    PSUM[0:256]; one extra PE matmul (ones^T @ probs) drops the per-
    j-tile sums of exp into PSUM[256:256+nj] of the same tile, so a
    single ACT copy + DMA ship both and the Exp needs no accumulator
Exp+matmuls run two chunks behind the scores, the PSUM evacuation
three chunks behind — otherwise the in-order ACT/PE queues park on
cross-engine dependencies and the pipeline serializes at chunk
latency. PE work bunches into >=3 us stretches, which also keeps the
tensor engine at its full 2.4 GHz p-state in the cost model.
Outputs per core: chat [NCH, 272] (context ++ per-j-tile exp sums) and
nmo [1, NCH] (minus chunk max). Host combine per batch:
c = sum_k w_k chat_k / sum_k w_k Z_k with w_k = exp(m_k - M).

Environment pitfalls avoided (discovered empirically on this device):
InstTensorTensorReduce faults the DVE (NRT_EXEC_UNIT_UNRECOVERABLE);
scalar_tensor_tensor gets no DVE perf modes in the cost model; InstPool
and TensorScalarPtr-with-accum are rejected on the Pool engine by
neuronxcc; plain tensor_scalar with accum_out needs op1/scalar2 set.
The Tile kernel-tail semaphore RANGE_CLEAR is replaced by a
drain+barrier-only tail (_tail_no_semclear).
"""

import sys
from contextlib import ExitStack

import numpy as np

sys.path.insert(0, "/opt/trn_rl_repo")

import concourse.bacc as bacc
import concourse.bass as bass
import concourse.bass_isa as bass_isa
import concourse.mybir as mybir
import concourse.tile as tile
from concourse.bass_utils import run_bass_kernel_spmd
from concourse.tile import ScopedClock


def _tail_no_semclear(self, tick_clock, wait_clock):
    """Tile's kernel-tail normally drains, barriers, then issues a GPSIMD
    dma_reset + EVENT_SEMAPHORE_RANGE_CLEAR over every sem it allocated.
    NRT resets semaphore state between executions, so drain + barrier alone
    is sufficient under the one-shot PJRT execution used here."""
    drain_inst = self.nc.sync.drain()
    wait_clock.add_sem_waits(
        drain_inst.ins, ScopedClock({None: tick_clock.global_clock})
    )
    self.nc.all_engine_barrier()
    popped = self.nc._tile_sem_poison_stack.pop()
    assert popped is self._sem_poison


tile.TileContext._drain_and_barrier = _tail_no_semclear

B, T, D = 32, 8192, 256
N_CORES = 8
B_LOC = B // N_CORES      # 4 batches per core
P = 128                   # partitions
NJB = T // P              # 64 j-tiles (128 t each) per batch

# chunk sizes in j-tiles per local batch, in schedule order: 1024-t chunks
# at the core's head and tail for fast pipeline fill/drain
BATCH_NJS = [
    [8, 8, 16, 16, 16],
    [16, 16, 16, 16],
    [16, 16, 16, 16],
    [16, 16, 16, 8, 8],
]
# (multiply j-tiles on GPSIMD, reduce j-tiles on ACT) per chunk size,
# from the engine-balance LP at each size's DMA-pace budget
SPLITS = {16: (5, 4), 8: (2, 1)}

CHUNKS = []  # (b, j0, nj)
for _b, _njs in enumerate(BATCH_NJS):
    assert sum(_njs) == NJB
    _j0 = 0
    for _nj in _njs:
        CHUNKS.append((_b, _j0, _nj))
        _j0 += _nj
NCH = len(CHUNKS)

F16 = mybir.dt.float16
F32 = mybir.dt.float32


def _build_nc():
    nc = bacc.Bacc(
        "TRN2",
        target_bir_lowering=False,
        debug=False,
        enable_asserts=False,
        num_devices=N_CORES,
    )
    enc = nc.dram_tensor("enc", [B_LOC, T, D], F16, kind="ExternalInput")
    dec = nc.dram_tensor("dec", [B_LOC, D], F16, kind="ExternalInput")
    # per chunk: [0:D] unnormalized context, [D:D+16] per-j-tile sums of
    # exp from the PE ones-row matmul (host sums them into Z)
    chat = nc.dram_tensor("chat", [NCH, D + 16], F32, kind="ExternalOutput")
    nmo = nc.dram_tensor("nmo", [1, NCH], F32, kind="ExternalOutput")

    enc_ap = enc.ap()
    dec_ap = dec.ap()

    def enc_chunk_src(b, j0, nj):
        # [p, j, d] with t = j0*128 + p*nj + j: per-partition contiguous run
        return bass.AP(
            tensor=enc_ap.tensor,
            offset=enc_ap.offset + (b * T + j0 * P) * D,
            ap=[[nj * D, P], [D, nj], [1, D]],
        )

    with tile.TileContext(nc) as tc, ExitStack() as ctx:
        st_pool = ctx.enter_context(tc.tile_pool(name="st", bufs=8))
        prod_pool = ctx.enter_context(tc.tile_pool(name="prod", bufs=3))
        small = ctx.enter_context(tc.tile_pool(name="small", bufs=8))
        stats = ctx.enter_context(tc.tile_pool(name="stats", bufs=1))
        psum_c = ctx.enter_context(tc.tile_pool(name="psc", bufs=8, space="PSUM"))

        # dec[b] replicated across partitions, all 4 batches in one DMA
        # (issued after the first enc chunk so the bus starts on enc sooner)
        dec_all = stats.tile([P, B_LOC, D], F16, tag="dec_all")
        dec_src = bass.AP(
            tensor=dec_ap.tensor,
            offset=dec_ap.offset,
            ap=[[0, P], [D, B_LOC], [1, D]],
        )

        # persistent stats tiles (written per-chunk as columns)
        negSM = stats.tile([P, NCH], F32, tag="negSM")  # -chunk max (bcast)
        ones_col = stats.tile([P, 1], F16, tag="ones_col")
        nc.vector.memset(ones_col, 1.0)
        # rotating throwaway outputs for the reduce ops: a single shared
        # buffer would chain consecutive reduces through WAW semaphores
        junk_pool = ctx.enter_context(tc.tile_pool(name="junk", bufs=6))

        ps_t = {}

        def all_reduce_max(k):
            # the m-reduce lives here, a full chunk after the scores were
            # issued, so ACT-copy jitter never stalls the DVE stream on it
            _, S, nj = state[k]
            m = small.tile([P, 1], F32, tag="m")
            nc.vector.tensor_reduce(
                out=m, in_=S, axis=mybir.AxisListType.X, op=mybir.AluOpType.max
            )
            mar = small.tile([P, 1], F32, tag="mar")
            nc.gpsimd.partition_all_reduce(
                mar, m, channels=P, reduce_op=bass_isa.ReduceOp.max
            )
            nc.gpsimd.tensor_scalar_mul(
                out=negSM[:, k : k + 1], in0=mar, scalar1=-1.0
            )

        def exp_and_matmul(k):
            st, S, nj = state[k]
            probs = small.tile([P, nj], F16, tag="probs")
            nc.scalar.activation(
                out=probs,
                in_=S,
                func=mybir.ActivationFunctionType.Exp,
                bias=negSM[:, k : k + 1],
                scale=1.0,
            )
            ps = psum_c.tile([1, D + 16], F32, tag="ps")
            for j in range(nj):
                nc.tensor.matmul(
                    out=ps[:, 0:D],
                    lhsT=probs[:, j : j + 1],
                    rhs=st[:, j, :],
                    start=(j == 0),
                    stop=(j == nj - 1),
                )
            # Z on the idle PE: ones^T @ probs = per-j-tile sums of exp,
            # landed in the same PSUM tile so one copy + one DMA ship both
            nc.tensor.matmul(
                out=ps[:, D : D + nj],
                lhsT=ones_col,
                rhs=probs,
                start=True,
                stop=True,
            )
            ps_t[k] = ps

        def store_ctx(k):
            csb = small.tile([1, D + 16], F32, tag="csb")
            nc.scalar.activation(
                out=csb, in_=ps_t[k],
                func=mybir.ActivationFunctionType.Copy, bias=0.0, scale=1.0,
            )
            nc.scalar.dma_start(out=chat.ap()[k : k + 1, :], in_=csb)

        state = {}
        for k, (b, j0, nj) in enumerate(CHUNKS):
            n_pool, n_act = SPLITS[nj]
            n_dve_mul = nj - n_pool
            st = st_pool.tile([P, nj, D], F16, tag="st")
            nc.sync.dma_start(out=st, in_=enc_chunk_src(b, j0, nj))
            if k == 0:
                nc.sync.dma_start(out=dec_all, in_=dec_src)

            dec_b3d = dec_all[:, b, :].rearrange("p (u d) -> p u d", u=1)
            # separate early multiply for the ACT-bound j-tiles so the ACT
            # reduce chain ends before DVE's and the m-reduce never stalls
            prod_a = prod_pool.tile([P, n_act, D], F16, tag="prod_a")
            nc.vector.tensor_tensor(
                out=prod_a,
                in0=st[:, 0:n_act, :],
                in1=dec_b3d.to_broadcast([P, n_act, D]),
                op=mybir.AluOpType.mult,
            )
            prod_d = prod_pool.tile([P, n_dve_mul - n_act, D], F16, tag="prod_d")
            nc.vector.tensor_tensor(
                out=prod_d,
                in0=st[:, n_act:n_dve_mul, :],
                in1=dec_b3d.to_broadcast([P, n_dve_mul - n_act, D]),
                op=mybir.AluOpType.mult,
            )
            prod_p = prod_pool.tile([P, n_pool, D], F16, tag="prod_p")
            nc.gpsimd.tensor_tensor(
                out=prod_p,
                in0=st[:, n_dve_mul:nj, :],
                in1=dec_b3d.to_broadcast([P, n_pool, D]),
                op=mybir.AluOpType.mult,
            )
            # all-reduce of the PREVIOUS chunk: issued after this chunk's
            # GPSIMD multiply so it never parks at the head of the in-order
            # Pool queue blocking that multiply (its m input is long ready)
            if k >= 1:
                all_reduce_max(k - 1)

            # ACT reduces the first j-tiles (from prod_d, ready earliest) so
            # its scores land before DVE's and the m-reduce never parks
            S = small.tile([P, nj], F32, tag="S")
            for j in range(n_act):
                junk_a = junk_pool.tile([P, D], F16, tag="junk_a")
                nc.scalar.activation(
                    out=junk_a, in_=prod_a[:, j, :],
                    func=mybir.ActivationFunctionType.Copy,
                    bias=0.0, scale=1.0,
                    accum_out=S[:, j : j + 1],
                )
            for j in range(n_act, nj):
                if j < n_dve_mul:
                    src = prod_d[:, j - n_act, :]
                else:
                    src = prod_p[:, j - n_dve_mul, :]
                junk_d = junk_pool.tile([P, D], F16, tag="junk_d")
                nc.vector.tensor_scalar(
                    out=junk_d, in0=src, scalar1=1.0, scalar2=0.0,
                    op0=mybir.AluOpType.mult, op1=mybir.AluOpType.add,
                    accum_out=S[:, j : j + 1],
                )

            state[k] = (st, S, nj)

            # software pipelining, two chunks deep for the Exp: with a
            # one-chunk shift the pipeline is paced by the latency cycle
            # m(k) -> all-reduce -> Exp(k) -> [in-order ACT] -> score
            # copies(k+1) -> m(k+1); at two chunks the ACT queue reaches the
            # next chunk's score copies before the Exp that waits on the
            # all-reduce, and the cycle spans two periods instead of one
            if k >= 2:
                exp_and_matmul(k - 2)
            if k >= 3:
                store_ctx(k - 3)
        all_reduce_max(NCH - 1)
        store_ctx(NCH - 3)
        exp_and_matmul(NCH - 2)
        exp_and_matmul(NCH - 1)
        store_ctx(NCH - 2)
        store_ctx(NCH - 1)

        nc.scalar.dma_start(out=nmo.ap(), in_=negSM[0:1, :])

    nc.compile()
    return nc


_NC_CACHE = None


def _get_nc():
    global _NC_CACHE
    if _NC_CACHE is None:
        _NC_CACHE = _build_nc()
    return _NC_CACHE


def run_on_cores(enc_np: np.ndarray, dec_np: np.ndarray, trace: bool = False):
    """Returns (out [32, 256] f32, BassKernelResults)."""
    nc = _get_nc()
    enc16 = enc_np.astype(np.float16)
    dec16 = dec_np.astype(np.float16)
    in_maps = [
        {
            "enc": np.ascontiguousarray(enc16[c * B_LOC : (c + 1) * B_LOC]),
            "dec": np.ascontiguousarray(dec16[c * B_LOC : (c + 1) * B_LOC]),
        }
        for c in range(N_CORES)
    ]
    res = run_bass_kernel_spmd(nc, in_maps, list(range(N_CORES)), trace=trace)

    out = np.empty((B, D), dtype=np.float64)
    for c in range(N_CORES):
        r = res.results[c]
        chat = np.asarray(r["chat"], dtype=np.float64)   # [NCH, D+16]
        mm = -np.asarray(r["nmo"], dtype=np.float64)[0]  # [NCH] chunk maxes
        # only the first nj z-columns of each chunk are written on device
        z = np.array([chat[k, D : D + nj].sum() for k, (_, _, nj) in enumerate(CHUNKS)])
        chat = chat[:, :D]
        for lb in range(B_LOC):
            ks = [k for k, (bb, _, _) in enumerate(CHUNKS) if bb == lb]
            m_k, z_k, c_k = mm[ks], z[ks], chat[ks]
            M = m_k.max()
            w = np.exp(m_k - M)
            out[c * B_LOC + lb] = (w[:, None] * c_k).sum(0) / (w * z_k).sum()
    return out.astype(np.float32), res


def kernel(enc_hid_states, dec_hid):
    enc_np = np.asarray(enc_hid_states, dtype=np.float32)
    dec_np = np.asarray(dec_hid, dtype=np.float32)
    out, _ = run_on_cores(enc_np, dec_np, trace=False)
    return out
